# revision 35
# baseline (speedup 1.0000x reference)
"""Additive (Bahdanau) attention on 8 TRN2 NeuronCores, data-parallel over batch.

Per core (one batch b):
  qf = queries @ W_q            [Q, H]   (computed transposed: qfT [H, Q])
  kf = keys @ W_k               [K, H]   (kfT [H, K])
  scores[q, k] = sum_h w_v[h] * tanh(qf[q, h] + kf[k, h])
  out = softmax_k(scores) @ values

Engine mapping:
  - VectorE: feat[h, k] = kfT + qfT[:, q] adds (bf16 out for the 4x DVE mode),
    psum->sbuf score drains, final normalization.
  - ScalarE: tanh over big [128, GQ*256] tiles (the throughput wall: 16.7M
    tanh/core at 1 elem/lane/cycle @ 1.2 GHz ~ 109us floor), exp with accum_out
    for the softmax denominator (max-subtraction skipped: |scores| <= sum|w_v|
    ~ 8, safely inside fp32 exp range).
  - TensorE: input transposes, projections, the w_v reduction as 4-way
    column-tiled mat-vecs over bf16 moving data (fp32 moving costs 4
    cycles/column on TRN2, bf16 costs 1), and attention @ V.
  - Scores land at psum partitions {0,32,64,96} (one strip per query pair,
    N=512 per matmul), are drained to SBUF, then compacted by an SBUF->SBUF
    DMA into a [q-pair, (q%2, k)] layout so softmax uses all 128 partitions.
"""

import functools
import sys

import numpy as np

sys.path.insert(0, "/opt/trn_rl_repo")

import concourse.bass as bass  # noqa: E402
import concourse.tile as tile  # noqa: E402
from concourse import bacc, mybir  # noqa: E402
from concourse.bass_utils import run_bass_kernel_spmd  # noqa: E402
from concourse.masks import make_identity  # noqa: E402

B, Q, K, D, H, DV = 8, 256, 256, 256, 256, 512
P = 128
GQ = 16         # queries per score sub-group (fixed: 8 pairs x 2 banks)
TGQ = 16        # queries per tanh/adds group (16 or 32)
XFUSE = 0       # of each group's GQ queries, how many use the fused bias-tanh path
TANH_SPLIT = 1  # activations per (chunk, group) big-tanh (overlap granularity)
DRAIN_MODE = "dve2"  # "dve2": DVE copy drains + end exp/accum; "act", "dve", "alt"
DMA_Q = "sync"  # queue for compaction DMAs: "sync", "scalar", "gpsimd", "alt"
GPS_ADDS = 0    # how many of each group's GQ adds (per chunk) go to GPSIMD
SKEW = 0        # software-pipeline the drain by one group
STAGE_F32 = 0   # stage/compaction in f32 (v2 behavior) instead of bf16
CASTS_GPS = 1   # input bf16 casts on gpsimd instead of DVE
TRUNC = 0       # 0 full; 1 no softmax/AV; 2 no drains; 3 adds+tanh only; 4 adds only
MM_ORDER = "jpair"  # "pair" | "jpair" (weights shared across banks) | "pass"
BUFS = dict(featp=2, tanhp=2, stagep=3, etp=2, psA=2, psS=2, psV=2)
NG = Q // GQ    # number of groups
F32 = mybir.dt.float32
BF16 = mybir.dt.bfloat16
AF = mybir.ActivationFunctionType
N_CORES = 8


def build_nc(dbg=False, reps=1):
    assert not (dbg and reps != 1)
    nc = bacc.Bacc("TRN2", target_bir_lowering=False, debug=False)

    q_ext = nc.declare_dram_parameter("queries", [Q, D], F32, isOutput=False)
    k_ext = nc.declare_dram_parameter("keys", [K, D], F32, isOutput=False)
    v_ext = nc.declare_dram_parameter("values", [K, DV], F32, isOutput=False)
    wq_ext = nc.declare_dram_parameter("W_q", [D, H], F32, isOutput=False)
    wk_ext = nc.declare_dram_parameter("W_k", [D, H], F32, isOutput=False)
    wv_ext = nc.declare_dram_parameter("w_v", [H], F32, isOutput=False)
    out_ext = nc.declare_dram_parameter("out", [Q, DV], F32, isOutput=True)
    dbg_ext = {}
    if dbg:
        dbg_ext["qfT"] = nc.declare_dram_parameter("dbg_qfT", [2, P, Q], F32, isOutput=True)
        dbg_ext["scoresD"] = nc.declare_dram_parameter("dbg_scoresD", [P, 2, K], F32, isOutput=True)
        dbg_ext["z"] = nc.declare_dram_parameter("dbg_z", [P, 2], F32, isOutput=True)
        dbg_ext["stage"] = nc.declare_dram_parameter("dbg_stage", [P, 2, 512], F32, isOutput=True)

    with tile.TileContext(nc) as tc:
        with (
            tc.tile_pool(name="consts", bufs=1) as consts,
            tc.tile_pool(name="io", bufs=1) as io,
            tc.tile_pool(name="work", bufs=1) as work,
            tc.tile_pool(name="featp", bufs=BUFS["featp"]) as featp,
            tc.tile_pool(name="tanhp", bufs=BUFS["tanhp"]) as tanhp,
            tc.tile_pool(name="stagep", bufs=BUFS["stagep"]) as stagep,
            tc.tile_pool(name="etp", bufs=BUFS["etp"]) as etp,
            tc.tile_pool(name="psA", bufs=BUFS["psA"], space=bass.MemorySpace.PSUM) as psA,
            tc.tile_pool(name="psS", bufs=BUFS["psS"], space=bass.MemorySpace.PSUM) as psS,
            tc.tile_pool(name="psV", bufs=BUFS["psV"], space=bass.MemorySpace.PSUM) as psV,
        ):
            ident = consts.tile([P, P], F32)
            make_identity(nc, ident)
            ident_bf = consts.tile([P, P], BF16)
            make_identity(nc, ident_bf)
            ident = (ident, ident_bf)
            pools = dict(consts=consts, io=io, work=work, featp=featp,
                         tanhp=tanhp, stagep=stagep, etp=etp,
                         psA=psA, psS=psS, psV=psV)
            exts = dict(q=q_ext, k=k_ext, v=v_ext, wq=wq_ext, wk=wk_ext,
                        wv=wv_ext, out=out_ext)
            for _rep in range(reps):
                _kernel_body(nc, pools, exts, ident, dbg_ext)

    nc.compile()
    return nc


def _kernel_body(nc, pools, exts, ident, dbg_ext):
    io, work, consts = pools["io"], pools["work"], pools["consts"]
    featp, tanhp, stagep, etp = (pools["featp"], pools["tanhp"],
                                 pools["stagep"], pools["etp"])
    psA, psS, psV = pools["psA"], pools["psS"], pools["psV"]
    ident, ident_bf = ident
    dbg = bool(dbg_ext)

    # ---- input loads ----
    qin, kin, v_sb, wq_sb, wk_sb = [], [], [], [], []
    for t in range(2):
        qt = io.tile([P, D], F32, name=f"qin{t}", tag=f"qin{t}")
        nc.sync.dma_start(out=qt, in_=exts["q"][t * P:(t + 1) * P, :])
        qin.append(qt)
        kt = io.tile([P, D], F32, name=f"kin{t}", tag=f"kin{t}")
        nc.sync.dma_start(out=kt, in_=exts["k"][t * P:(t + 1) * P, :])
        kin.append(kt)
        vt = io.tile([P, DV], F32, name=f"vin{t}", tag=f"vin{t}")
        nc.sync.dma_start(out=vt, in_=exts["v"][t * P:(t + 1) * P, :])
        v_sb.append(vt)
        wqt = io.tile([P, H], F32, name=f"wq{t}", tag=f"wq{t}")
        nc.sync.dma_start(out=wqt, in_=exts["wq"][t * P:(t + 1) * P, :])
        wq_sb.append(wqt)
        wkt = io.tile([P, H], F32, name=f"wk{t}", tag=f"wk{t}")
        nc.sync.dma_start(out=wkt, in_=exts["wk"][t * P:(t + 1) * P, :])
        wk_sb.append(wkt)

    # bf16 casts of matmul operands
    v_bf, wq_bf, wk_bf = [], [], []
    for t in range(2):
        vb = io.tile([P, DV], BF16, name=f"vbf{t}", tag=f"vbf{t}")
        (nc.gpsimd if CASTS_GPS else nc.vector).tensor_copy(out=vb, in_=v_sb[t])
        v_bf.append(vb)
        wqb = io.tile([P, H], BF16, name=f"wqbf{t}", tag=f"wqbf{t}")
        (nc.gpsimd if CASTS_GPS else nc.vector).tensor_copy(out=wqb, in_=wq_sb[t])
        wq_bf.append(wqb)
        wkb = io.tile([P, H], BF16, name=f"wkbf{t}", tag=f"wkbf{t}")
        (nc.gpsimd if CASTS_GPS else nc.vector).tensor_copy(out=wkb, in_=wk_sb[t])
        wk_bf.append(wkb)

    wv_sb = consts.tile([P, 2], F32, name="wv_sb", tag="wv_sb")
    for c in range(2):
        nc.sync.dma_start(out=wv_sb[:, c:c + 1], in_=exts["wv"][c * P:(c + 1) * P])
    # w_v chunks replicated to 32 bf16 columns: stationary for the matvecs
    wv_rep = consts.tile([P, 2, 32], BF16, name="wv_rep", tag="wv_rep")
    for c in range(2):
        nc.gpsimd.tensor_copy(
            out=wv_rep[:, c, :],
            in_=wv_sb[:, c:c + 1].broadcast_to((P, 32)),
        )

    # ---- transpose queries/keys -> bf16 [d_sub, q] ----
    qT = [work.tile([P, Q], BF16, name=f"qTd{dc}", tag=f"qTd{dc}") for dc in range(2)]
    kT = [work.tile([P, K], BF16, name=f"kTd{dc}", tag=f"kTd{dc}") for dc in range(2)]
    for src_tiles, dstT in ((qin, qT), (kin, kT)):
        for dc in range(2):
            for t in range(2):
                tp = psA.tile([P, 256], F32, name="ps_tr", tag="ps_m")
                nc.tensor.matmul(
                    tp[:, 0:P],
                    lhsT=src_tiles[t][:, dc * P:(dc + 1) * P],
                    rhs=ident,
                    is_transpose=True,
                    start=True,
                    stop=True,
                )
                nc.vector.tensor_copy(dstT[dc][:, t * P:(t + 1) * P], tp[:, 0:P])

    # ---- projections: qfT[c] f32 (bias source), kfB[c] bf16 (add source) ----
    qfT, kfB = [], []
    for name, srcT, w_tiles in (("qf", qT, wq_bf), ("kf", kT, wk_bf)):
        for c in range(2):
            pp = psA.tile([P, 256], F32, name="ps_pr", tag="ps_m")
            for dc in range(2):
                nc.tensor.matmul(
                    pp,
                    lhsT=w_tiles[dc][:, c * P:(c + 1) * P],
                    rhs=srcT[dc],
                    start=(dc == 0),
                    stop=(dc == 1),
                )
            if name == "qf":
                t_sb = work.tile([P, Q], F32, name=f"qfT{c}", tag=f"qfT{c}")
                nc.vector.tensor_copy(t_sb, pp)
                qfT.append(t_sb)
            else:
                t_bf = work.tile([P, K], BF16, name=f"kfB{c}", tag=f"kfB{c}")
                nc.vector.tensor_copy(t_bf, pp)
                kfB.append(t_bf)

    if dbg:
        for c in range(2):
            nc.sync.dma_start(out=dbg_ext["qfT"][c], in_=qfT[c])

    # ---- main loop over query groups (drain software-pipelined one group) ----
    # eD[p, j0, k] = exp(scores[2p + j0, k]); exp happens in the psum drain
    eD = work.tile([P, 2, K], BF16, name="eD", tag="eD")
    pend = None  # (g, sc_ps) awaiting drain

    def drain(g, sc_ps):
        # drain = exp: every psum row holds real scores (32 replicated rows
        # per strip). Groups alternate between an ACT exp-drain (e values) and
        # a DVE copy-drain (raw scores, exp'd once at the end) to balance the
        # two engines; copy-drained groups write the dense tile sD instead.
        is_act = DRAIN_MODE == "act" or (DRAIN_MODE == "alt" and g % 2 == 0)
        if DRAIN_MODE == "dve2":
            is_act = False
        st = stagep.tile([P, 2, 512], F32 if STAGE_F32 else BF16,
                         name="stage", tag="stage")
        if is_act:
            nc.scalar.activation(out=st, in_=sc_ps, func=AF.Exp)
        else:
            nc.vector.tensor_copy(out=st, in_=sc_ps)
        if dbg and g == 0:
            nc.gpsimd.dma_start(out=dbg_ext["stage"][:], in_=st)
        # compact rows {0,32,64,96} -> eD/sD[8g:8g+8]; pair p=4b+j lands at
        # partition 8g+p holding (q_even | q_odd) halves. One DMA per bank b
        # (SBUF DMA APs may only cross partitions on their first dim); the
        # two HWDGE queues (sync, act) alternate by group.
        dst = eD if is_act else sD
        dq = {"sync": nc.sync, "scalar": nc.scalar, "gpsimd": nc.gpsimd}.get(
            DMA_Q, [nc.sync, nc.scalar][g % 2])
        for b in range(2):
            dq.dma_start(
                out=dst[8 * g + 4 * b:8 * g + 4 * b + 4, :, :],
                in_=st[0:P:32, b, :],
            )

    sD = work.tile([P, 2, K], F32 if STAGE_F32 else BF16, name="sD", tag="sD")
    tanh_big = None
    for g in range(NG):
        # adds + tanh emitted once per TGQ queries; score sub-groups are 16
        if (g * GQ) % TGQ == 0:
            tanh_big = []
            for c in range(2):
                nv = TGQ - XFUSE
                th = tanhp.tile([P, TGQ * K], BF16, name=f"tanh{c}", tag=f"tanh{c}")
                if nv:
                    feat = featp.tile([P, nv * K], BF16, name=f"feat{c}", tag=f"feat{c}")
                    for qi in range(nv):
                        q = (g * GQ // TGQ) * TGQ + qi
                        eng = nc.gpsimd if qi < GPS_ADDS else nc.vector
                        eng.tensor_scalar_add(
                            out=feat[:, qi * K:(qi + 1) * K],
                            in0=kfB[c],
                            scalar1=qfT[c][:, q:q + 1],
                        )
                    step = (nv * K) // TANH_SPLIT
                    for si in range(TANH_SPLIT if TRUNC < 4 else 0):
                        nc.scalar.activation(
                            out=th[:, si * step:(si + 1) * step],
                            in_=feat[:, si * step:(si + 1) * step],
                            func=AF.Tanh,
                        )
                for qi in range(nv, TGQ):
                    q = (g * GQ // TGQ) * TGQ + qi
                    nc.scalar.activation(
                        out=th[:, qi * K:(qi + 1) * K],
                        in_=kfB[c],
                        func=AF.Tanh,
                        bias=qfT[c][:, q:q + 1],
                    )
                tanh_big.append(th)
        off = (g * GQ) % TGQ
        tanh_t = [tb[:, off * K:(off + GQ) * K] for tb in tanh_big]

        if TRUNC >= 3:
            continue
        # scores: pair p=4b+j covers queries (16g+2p, 16g+2p+1); strip j,
        # psum bank b, rows 32j..32j+31, one N=512 matmul per (pair, chunk)
        sc_ps = psS.tile([P, 2, 512], F32, name="sc_ps", tag="sc")
        if MM_ORDER == "jpair":
            # per strip: w0 once for both banks, then w1 for both banks.
            # Bank-granular has_written clears make this safe: each bank sees
            # start -> accumulate before any other start touches it.
            for j in range(4):
                for c in range(2):
                    for b in range(2):
                        p = 4 * b + j
                        o = sc_ps[32 * j:32 * j + 32, b, :]
                        mv = slice(2 * p * K, (2 * p + 2) * K)
                        nc.tensor.matmul(
                            o, lhsT=wv_rep[:, c, :], rhs=tanh_t[c][:, mv],
                            start=(c == 0), stop=(c == 1),
                            tile_position=(0, 32 * j),
                        )
        elif MM_ORDER == "pass":
            for c in range(2):
                for j in range(4):
                    for b in range(2):
                        p = 4 * b + j
                        o = sc_ps[32 * j:32 * j + 32, b, :]
                        mv = slice(2 * p * K, (2 * p + 2) * K)
                        nc.tensor.matmul(
                            o, lhsT=wv_rep[:, c, :], rhs=tanh_t[c][:, mv],
                            start=(c == 0), stop=(c == 1),
                            tile_position=(0, 32 * j),
                        )
        else:
            for b in range(2):
                for j in range(4):
                    p = 4 * b + j
                    o = sc_ps[32 * j:32 * j + 32, b, :]
                    mv = slice(2 * p * K, (2 * p + 2) * K)
                    nc.tensor.matmul(
                        o, lhsT=wv_rep[:, 0, :], rhs=tanh_t[0][:, mv],
                        start=True, stop=False, tile_position=(0, 32 * j),
                    )
                    nc.tensor.matmul(
                        o, lhsT=wv_rep[:, 1, :], rhs=tanh_t[1][:, mv],
                        start=False, stop=True, tile_position=(0, 32 * j),
                    )

        if TRUNC >= 2:
            continue
        if SKEW:
            if pend is not None:
                drain(*pend)
            pend = (g, sc_ps)
        else:
            drain(g, sc_ps)
    if pend is not None and TRUNC < 2:
        drain(*pend)

    # exp the copy-drained groups' scores (odd groups live at partition
    # ranges [8g, 8g+8) of sD); finish them into eD in two activation calls
    # covering the odd-group partition stripes via a strided partition AP is
    # not possible on ACT, so do one activation per odd group stripe.
    if DRAIN_MODE == "dve2":
        pass  # exp+accum happens in the softmax section below
    elif DRAIN_MODE != "act":
        gs = range(1, NG, 2) if DRAIN_MODE == "alt" else range(NG)
        for g in gs:
            nc.scalar.activation(
                out=eD[8 * g:8 * g + 8, :, :],
                in_=sD[8 * g:8 * g + 8, :, :],
                func=AF.Exp,
            )


    if TRUNC >= 1:
        # still emit an output so the graph has one
        dummy = work.tile([P, DV], F32, name="dummy_out", tag="outF0")
        nc.vector.memset(dummy, 0.0)
        ov = exts["out"][:].rearrange("(p two) v -> p two v", two=2)
        nc.sync.dma_start(out=ov[:, 0, :], in_=dummy)
        return

    # ---- softmax denominator from the dense e tile ----
    e = eD
    zsum = work.tile([P, 2], F32, name="zsum", tag="zsum")
    if DRAIN_MODE == "dve2":
        for j0 in range(2):
            nc.scalar.activation(
                out=eD[:, j0, :],
                in_=sD[:, j0, :],
                func=AF.Exp,
                accum_out=zsum[:, j0:j0 + 1],
            )
    else:
        for j0 in range(2):
            nc.vector.reduce_sum(
                out=zsum[:, j0:j0 + 1], in_=eD[:, j0, :], axis=mybir.AxisListType.X
            )
    zr = work.tile([P, 2], F32, name="zr", tag="zr")
    nc.vector.reciprocal(zr, zsum)
    if dbg:
        nc.gpsimd.dma_start(out=dbg_ext["scoresD"][:], in_=eD)
        nc.sync.dma_start(out=dbg_ext["z"][:], in_=zsum)

    # ---- attention @ V ----
    out_view = exts["out"][:].rearrange("(p two) v -> p two v", two=2)
    for j0 in range(2):
        av_ps = psV.tile([P, DV], F32, name="av_ps", tag="av")
        for kh in range(2):
            tp = psA.tile([P, 256], BF16, name="ps_et", tag="ps_m")
            nc.tensor.matmul(
                tp[:, 0:P],
                lhsT=e[:, j0, kh * P:(kh + 1) * P],
                rhs=ident_bf,
                is_transpose=True,
                start=True,
                stop=True,
            )
            eT = etp.tile([P, P], BF16, name="eT", tag="eT")
            nc.vector.tensor_copy(eT, tp[:, 0:P])
            nc.tensor.matmul(
                av_ps, lhsT=eT, rhs=v_bf[kh],
                start=(kh == 0), stop=(kh == 1),
            )
        outF = work.tile([P, DV], F32, name=f"outF{j0}", tag=f"outF{j0}")
        nc.vector.tensor_scalar_mul(outF, av_ps, zr[:, j0:j0 + 1])
        nc.sync.dma_start(out=out_view[:, j0, :], in_=outF)


@functools.lru_cache(maxsize=4)
def _get_nc(reps=1):
    return build_nc(reps=reps)


def _in_maps(inputs):
    in_maps = []
    for i in range(N_CORES):
        in_maps.append({
            "queries": np.ascontiguousarray(inputs["queries"][i], dtype=np.float32),
            "keys": np.ascontiguousarray(inputs["keys"][i], dtype=np.float32),
            "values": np.ascontiguousarray(inputs["values"][i], dtype=np.float32),
            "W_q": np.ascontiguousarray(inputs["W_q"], dtype=np.float32),
            "W_k": np.ascontiguousarray(inputs["W_k"], dtype=np.float32),
            "w_v": np.ascontiguousarray(inputs["w_v"], dtype=np.float32),
        })
    return in_maps


def _run(inputs, trace=False):
    nc = _get_nc()
    in_maps = _in_maps(inputs)
    res = run_bass_kernel_spmd(nc, in_maps, core_ids=list(range(N_CORES)), trace=trace)
    out = np.stack([res.results[i]["out"] for i in range(N_CORES)], axis=0)
    return out.astype(np.float32), res


def kernel(**inputs) -> np.ndarray:
    return _run(inputs)[0]


# revision 37
# speedup vs baseline: 1.0091x; 1.0091x over previous
"""Additive (Bahdanau) attention on 8 TRN2 NeuronCores, data-parallel over batch.

Per core (one batch b):
  qf = queries @ W_q            [Q, H]   (kept transposed: qfT [H, Q], fp32)
  kf = keys @ W_k               [K, H]   (kfT [H, K], bf16)
  scores[q, k] = sum_h w_v[h] * tanh(qf[q, h] + kf[k, h])
  out = softmax_k(scores) @ values

Engine mapping (per 16-query group, 16 groups):
  - VectorE: feat[h, k] = kfB + qfT[:, q] per-partition-scalar adds (bf16 for
    the fast DVE modes, one instr per (query, h-chunk)), the psum->sbuf score
    drains, and the final 1/Z normalization.
  - ScalarE: tanh over [128, 16*256] bf16 tiles - the throughput wall: 16.7M
    tanh/core at 1 elem/lane/cycle @ 1.2 GHz is a ~109 us floor - plus one
    exp-with-accum_out pass for softmax (max-subtraction skipped: |scores| <=
    sum|w_v| ~ 8, safely inside fp32 exp range).
  - TensorE: input transposes, projections, the w_v reduction as 4-way
    column-tiled mat-vecs over bf16 moving data (fp32 moving costs 4
    cycles/column on TRN2, bf16 costs 1), and attention @ V. Query pair
    p = 4b + j uses array column strip j (tile_position=(0, 32j), stationary =
    w_v chunk replicated to 32 cols) and psum bank b, one N=512 matmul per
    (pair, h-chunk); per strip w0 serves both banks before swapping to w1.
  - Scores land replicated on psum rows 32j..32j+31, are copy-drained to SBUF
    (bf16) and compacted by SBUF->SBUF DMAs into a dense [q-pair, (q%2, k)]
    layout so softmax and attention@V run on full 128-partition tiles.

Approximate cost-model timeline: ~144 us/core; ScalarE-bound.
"""

import functools
import sys

import numpy as np

sys.path.insert(0, "/opt/trn_rl_repo")

import concourse.bass as bass  # noqa: E402
import concourse.tile as tile  # noqa: E402
from concourse import bacc, mybir  # noqa: E402
from concourse.bass_utils import run_bass_kernel_spmd  # noqa: E402
from concourse.masks import make_identity  # noqa: E402

B, Q, K, D, H, DV = 8, 256, 256, 256, 256, 512
P = 128
GQ = 16         # queries per score sub-group (fixed: 8 pairs x 2 banks)
TGQ = 16        # queries per tanh/adds group (16 or 32)
XFUSE = 0       # of each group's GQ queries, how many use the fused bias-tanh path
TANH_SPLIT = 1  # activations per (chunk, group) big-tanh (overlap granularity)
DRAIN_MODE = "dve2"  # "dve2": DVE copy drains + end exp/accum; "act", "dve", "alt"
DMA_Q = "sync"  # queue for compaction DMAs: "sync", "scalar", "gpsimd", "alt"
GPS_ADDS = 0    # how many of each group's GQ adds (per chunk) go to GPSIMD
SKEW = 0        # software-pipeline the drain by one group
STAGE_F32 = 0   # stage/compaction in f32 (v2 behavior) instead of bf16
CASTS_GPS = 1   # input bf16 casts on gpsimd instead of DVE
TRUNC = 0       # 0 full; 1 no softmax/AV; 2 no drains; 3 adds+tanh only; 4 adds only
MM_ORDER = "jpair"  # "pair" | "jpair" (weights shared across banks) | "pass"
BUFS = dict(featp=2, tanhp=2, stagep=3, etp=2, psA=2, psS=2, psV=2)
NG = Q // GQ    # number of groups
F32 = mybir.dt.float32
BF16 = mybir.dt.bfloat16
AF = mybir.ActivationFunctionType
N_CORES = 8


def build_nc(dbg=False, reps=1):
    assert not (dbg and reps != 1)
    nc = bacc.Bacc("TRN2", target_bir_lowering=False, debug=False)

    q_ext = nc.declare_dram_parameter("queries", [Q, D], F32, isOutput=False)
    k_ext = nc.declare_dram_parameter("keys", [K, D], F32, isOutput=False)
    v_ext = nc.declare_dram_parameter("values", [K, DV], F32, isOutput=False)
    wq_ext = nc.declare_dram_parameter("W_q", [D, H], F32, isOutput=False)
    wk_ext = nc.declare_dram_parameter("W_k", [D, H], F32, isOutput=False)
    wv_ext = nc.declare_dram_parameter("w_v", [H], F32, isOutput=False)
    out_ext = nc.declare_dram_parameter("out", [Q, DV], F32, isOutput=True)
    dbg_ext = {}
    if dbg:
        dbg_ext["qfT"] = nc.declare_dram_parameter("dbg_qfT", [2, P, Q], F32, isOutput=True)
        dbg_ext["scoresD"] = nc.declare_dram_parameter("dbg_scoresD", [P, 2, K], F32, isOutput=True)
        dbg_ext["z"] = nc.declare_dram_parameter("dbg_z", [P, 2], F32, isOutput=True)
        dbg_ext["stage"] = nc.declare_dram_parameter("dbg_stage", [P, 2, 512], F32, isOutput=True)

    with tile.TileContext(nc) as tc:
        with (
            tc.tile_pool(name="consts", bufs=1) as consts,
            tc.tile_pool(name="io", bufs=1) as io,
            tc.tile_pool(name="work", bufs=1) as work,
            tc.tile_pool(name="featp", bufs=BUFS["featp"]) as featp,
            tc.tile_pool(name="tanhp", bufs=BUFS["tanhp"]) as tanhp,
            tc.tile_pool(name="stagep", bufs=BUFS["stagep"]) as stagep,
            tc.tile_pool(name="etp", bufs=BUFS["etp"]) as etp,
            tc.tile_pool(name="psA", bufs=BUFS["psA"], space=bass.MemorySpace.PSUM) as psA,
            tc.tile_pool(name="psS", bufs=BUFS["psS"], space=bass.MemorySpace.PSUM) as psS,
            tc.tile_pool(name="psV", bufs=BUFS["psV"], space=bass.MemorySpace.PSUM) as psV,
        ):
            ident = consts.tile([P, P], F32)
            make_identity(nc, ident)
            ident_bf = consts.tile([P, P], BF16)
            make_identity(nc, ident_bf)
            ident = (ident, ident_bf)
            pools = dict(consts=consts, io=io, work=work, featp=featp,
                         tanhp=tanhp, stagep=stagep, etp=etp,
                         psA=psA, psS=psS, psV=psV)
            exts = dict(q=q_ext, k=k_ext, v=v_ext, wq=wq_ext, wk=wk_ext,
                        wv=wv_ext, out=out_ext)
            for _rep in range(reps):
                _kernel_body(nc, pools, exts, ident, dbg_ext)

    nc.compile()
    return nc


def _kernel_body(nc, pools, exts, ident, dbg_ext):
    io, work, consts = pools["io"], pools["work"], pools["consts"]
    featp, tanhp, stagep, etp = (pools["featp"], pools["tanhp"],
                                 pools["stagep"], pools["etp"])
    psA, psS, psV = pools["psA"], pools["psS"], pools["psV"]
    ident, ident_bf = ident
    dbg = bool(dbg_ext)

    # ---- input loads ----
    qin, kin, v_sb, wq_sb, wk_sb = [], [], [], [], []
    for t in range(2):
        qt = io.tile([P, D], F32, name=f"qin{t}", tag=f"qin{t}")
        nc.sync.dma_start(out=qt, in_=exts["q"][t * P:(t + 1) * P, :])
        qin.append(qt)
        kt = io.tile([P, D], F32, name=f"kin{t}", tag=f"kin{t}")
        nc.sync.dma_start(out=kt, in_=exts["k"][t * P:(t + 1) * P, :])
        kin.append(kt)
        wqt = io.tile([P, H], F32, name=f"wq{t}", tag=f"wq{t}")
        nc.sync.dma_start(out=wqt, in_=exts["wq"][t * P:(t + 1) * P, :])
        wq_sb.append(wqt)
        wkt = io.tile([P, H], F32, name=f"wk{t}", tag=f"wk{t}")
        nc.sync.dma_start(out=wkt, in_=exts["wk"][t * P:(t + 1) * P, :])
        wk_sb.append(wkt)

    # bf16 casts of matmul operands
    v_bf, wq_bf, wk_bf = [], [], []
    for t in range(2):
        wqb = io.tile([P, H], BF16, name=f"wqbf{t}", tag=f"wqbf{t}")
        (nc.gpsimd if CASTS_GPS else nc.vector).tensor_copy(out=wqb, in_=wq_sb[t])
        wq_bf.append(wqb)
        wkb = io.tile([P, H], BF16, name=f"wkbf{t}", tag=f"wkbf{t}")
        (nc.gpsimd if CASTS_GPS else nc.vector).tensor_copy(out=wkb, in_=wk_sb[t])
        wk_bf.append(wkb)

    wv_sb = consts.tile([P, 2], F32, name="wv_sb", tag="wv_sb")
    for c in range(2):
        nc.sync.dma_start(out=wv_sb[:, c:c + 1], in_=exts["wv"][c * P:(c + 1) * P])
    # w_v chunks replicated to 32 bf16 columns: stationary for the matvecs
    wv_rep = consts.tile([P, 2, 32], BF16, name="wv_rep", tag="wv_rep")
    for c in range(2):
        nc.gpsimd.tensor_copy(
            out=wv_rep[:, c, :],
            in_=wv_sb[:, c:c + 1].broadcast_to((P, 32)),
        )

    # ---- transpose queries/keys -> bf16 [d_sub, q] ----
    qT = [work.tile([P, Q], BF16, name=f"qTd{dc}", tag=f"qTd{dc}") for dc in range(2)]
    kT = [work.tile([P, K], BF16, name=f"kTd{dc}", tag=f"kTd{dc}") for dc in range(2)]
    for src_tiles, dstT in ((qin, qT), (kin, kT)):
        for dc in range(2):
            for t in range(2):
                tp = psA.tile([P, 256], F32, name="ps_tr", tag="ps_m")
                nc.tensor.matmul(
                    tp[:, 0:P],
                    lhsT=src_tiles[t][:, dc * P:(dc + 1) * P],
                    rhs=ident,
                    is_transpose=True,
                    start=True,
                    stop=True,
                )
                nc.vector.tensor_copy(dstT[dc][:, t * P:(t + 1) * P], tp[:, 0:P])

    # ---- projections: qfT[c] f32 (bias source), kfB[c] bf16 (add source) ----
    qfT, kfB = [], []
    for name, srcT, w_tiles in (("qf", qT, wq_bf), ("kf", kT, wk_bf)):
        for c in range(2):
            pp = psA.tile([P, 256], F32, name="ps_pr", tag="ps_m")
            for dc in range(2):
                nc.tensor.matmul(
                    pp,
                    lhsT=w_tiles[dc][:, c * P:(c + 1) * P],
                    rhs=srcT[dc],
                    start=(dc == 0),
                    stop=(dc == 1),
                )
            if name == "qf":
                t_sb = work.tile([P, Q], F32, name=f"qfT{c}", tag=f"qfT{c}")
                nc.vector.tensor_copy(t_sb, pp)
                qfT.append(t_sb)
            else:
                t_bf = work.tile([P, K], BF16, name=f"kfB{c}", tag=f"kfB{c}")
                nc.vector.tensor_copy(t_bf, pp)
                kfB.append(t_bf)

    if dbg:
        for c in range(2):
            nc.sync.dma_start(out=dbg_ext["qfT"][c], in_=qfT[c])

    # values load + bf16 cast (only needed by the AV tail; off the head path)
    for t in range(2):
        vt = io.tile([P, DV], F32, name=f"vin{t}", tag=f"vin{t}")
        nc.sync.dma_start(out=vt, in_=exts["v"][t * P:(t + 1) * P, :])
        v_sb.append(vt)
        vb = io.tile([P, DV], BF16, name=f"vbf{t}", tag=f"vbf{t}")
        (nc.gpsimd if CASTS_GPS else nc.vector).tensor_copy(out=vb, in_=v_sb[t])
        v_bf.append(vb)

    # ---- main loop over query groups (drain software-pipelined one group) ----
    # eD[p, j0, k] = exp(scores[2p + j0, k]); exp happens in the psum drain
    eD = work.tile([P, 2, K], BF16, name="eD", tag="eD")
    pend = None  # (g, sc_ps) awaiting drain

    def drain(g, sc_ps):
        # drain = exp: every psum row holds real scores (32 replicated rows
        # per strip). Groups alternate between an ACT exp-drain (e values) and
        # a DVE copy-drain (raw scores, exp'd once at the end) to balance the
        # two engines; copy-drained groups write the dense tile sD instead.
        is_act = DRAIN_MODE == "act" or (DRAIN_MODE == "alt" and g % 2 == 0)
        if DRAIN_MODE == "dve2":
            is_act = False
        st = stagep.tile([P, 2, 512], F32 if STAGE_F32 else BF16,
                         name="stage", tag="stage")
        if is_act:
            nc.scalar.activation(out=st, in_=sc_ps, func=AF.Exp)
        else:
            nc.vector.tensor_copy(out=st, in_=sc_ps)
        if dbg and g == 0:
            nc.gpsimd.dma_start(out=dbg_ext["stage"][:], in_=st)
        # compact rows {0,32,64,96} -> eD/sD[8g:8g+8]; pair p=4b+j lands at
        # partition 8g+p holding (q_even | q_odd) halves. One DMA per bank b
        # (SBUF DMA APs may only cross partitions on their first dim); the
        # two HWDGE queues (sync, act) alternate by group.
        dst = eD if is_act else sD
        dq = {"sync": nc.sync, "scalar": nc.scalar, "gpsimd": nc.gpsimd}.get(
            DMA_Q, [nc.sync, nc.scalar][g % 2])
        for b in range(2):
            dq.dma_start(
                out=dst[8 * g + 4 * b:8 * g + 4 * b + 4, :, :],
                in_=st[0:P:32, b, :],
            )

    sD = work.tile([P, 2, K], F32 if STAGE_F32 else BF16, name="sD", tag="sD")
    tanh_big = None
    for g in range(NG):
        # adds + tanh emitted once per TGQ queries; score sub-groups are 16
        if (g * GQ) % TGQ == 0:
            tanh_big = []
            for c in range(2):
                nv = TGQ - XFUSE
                th = tanhp.tile([P, TGQ * K], BF16, name=f"tanh{c}", tag=f"tanh{c}")
                if nv:
                    feat = featp.tile([P, nv * K], BF16, name=f"feat{c}", tag=f"feat{c}")
                    for qi in range(nv):
                        q = (g * GQ // TGQ) * TGQ + qi
                        eng = nc.gpsimd if qi < GPS_ADDS else nc.vector
                        eng.tensor_scalar_add(
                            out=feat[:, qi * K:(qi + 1) * K],
                            in0=kfB[c],
                            scalar1=qfT[c][:, q:q + 1],
                        )
                    step = (nv * K) // TANH_SPLIT
                    for si in range(TANH_SPLIT if TRUNC < 4 else 0):
                        nc.scalar.activation(
                            out=th[:, si * step:(si + 1) * step],
                            in_=feat[:, si * step:(si + 1) * step],
                            func=AF.Tanh,
                        )
                for qi in range(nv, TGQ):
                    q = (g * GQ // TGQ) * TGQ + qi
                    nc.scalar.activation(
                        out=th[:, qi * K:(qi + 1) * K],
                        in_=kfB[c],
                        func=AF.Tanh,
                        bias=qfT[c][:, q:q + 1],
                    )
                tanh_big.append(th)
        off = (g * GQ) % TGQ
        tanh_t = [tb[:, off * K:(off + GQ) * K] for tb in tanh_big]

        if TRUNC >= 3:
            continue
        # scores: pair p=4b+j covers queries (16g+2p, 16g+2p+1); strip j,
        # psum bank b, rows 32j..32j+31, one N=512 matmul per (pair, chunk)
        sc_ps = psS.tile([P, 2, 512], F32, name="sc_ps", tag="sc")
        if MM_ORDER == "jpair":
            # per strip: w0 once for both banks, then w1 for both banks.
            # Bank-granular has_written clears make this safe: each bank sees
            # start -> accumulate before any other start touches it.
            for j in range(4):
                for c in range(2):
                    for b in range(2):
                        p = 4 * b + j
                        o = sc_ps[32 * j:32 * j + 32, b, :]
                        mv = slice(2 * p * K, (2 * p + 2) * K)
                        nc.tensor.matmul(
                            o, lhsT=wv_rep[:, c, :], rhs=tanh_t[c][:, mv],
                            start=(c == 0), stop=(c == 1),
                            tile_position=(0, 32 * j),
                        )
        elif MM_ORDER == "pass":
            for c in range(2):
                for j in range(4):
                    for b in range(2):
                        p = 4 * b + j
                        o = sc_ps[32 * j:32 * j + 32, b, :]
                        mv = slice(2 * p * K, (2 * p + 2) * K)
                        nc.tensor.matmul(
                            o, lhsT=wv_rep[:, c, :], rhs=tanh_t[c][:, mv],
                            start=(c == 0), stop=(c == 1),
                            tile_position=(0, 32 * j),
                        )
        else:
            for b in range(2):
                for j in range(4):
                    p = 4 * b + j
                    o = sc_ps[32 * j:32 * j + 32, b, :]
                    mv = slice(2 * p * K, (2 * p + 2) * K)
                    nc.tensor.matmul(
                        o, lhsT=wv_rep[:, 0, :], rhs=tanh_t[0][:, mv],
                        start=True, stop=False, tile_position=(0, 32 * j),
                    )
                    nc.tensor.matmul(
                        o, lhsT=wv_rep[:, 1, :], rhs=tanh_t[1][:, mv],
                        start=False, stop=True, tile_position=(0, 32 * j),
                    )

        if TRUNC >= 2:
            continue
        if SKEW:
            if pend is not None:
                drain(*pend)
            pend = (g, sc_ps)
        else:
            drain(g, sc_ps)
    if pend is not None and TRUNC < 2:
        drain(*pend)

    # exp the copy-drained groups' scores (odd groups live at partition
    # ranges [8g, 8g+8) of sD); finish them into eD in two activation calls
    # covering the odd-group partition stripes via a strided partition AP is
    # not possible on ACT, so do one activation per odd group stripe.
    if DRAIN_MODE == "dve2":
        pass  # exp+accum happens in the softmax section below
    elif DRAIN_MODE != "act":
        gs = range(1, NG, 2) if DRAIN_MODE == "alt" else range(NG)
        for g in gs:
            nc.scalar.activation(
                out=eD[8 * g:8 * g + 8, :, :],
                in_=sD[8 * g:8 * g + 8, :, :],
                func=AF.Exp,
            )


    if TRUNC >= 1:
        # still emit an output so the graph has one
        dummy = work.tile([P, DV], F32, name="dummy_out", tag="outF0")
        nc.vector.memset(dummy, 0.0)
        ov = exts["out"][:].rearrange("(p two) v -> p two v", two=2)
        nc.sync.dma_start(out=ov[:, 0, :], in_=dummy)
        return

    # ---- softmax denominator from the dense e tile ----
    e = eD
    zsum = work.tile([P, 2], F32, name="zsum", tag="zsum")
    if DRAIN_MODE == "dve2":
        for j0 in range(2):
            nc.scalar.activation(
                out=eD[:, j0, :],
                in_=sD[:, j0, :],
                func=AF.Exp,
                accum_out=zsum[:, j0:j0 + 1],
            )
    else:
        for j0 in range(2):
            nc.vector.reduce_sum(
                out=zsum[:, j0:j0 + 1], in_=eD[:, j0, :], axis=mybir.AxisListType.X
            )
    zr = work.tile([P, 2], F32, name="zr", tag="zr")
    nc.vector.reciprocal(zr, zsum)
    if dbg:
        nc.gpsimd.dma_start(out=dbg_ext["scoresD"][:], in_=eD)
        nc.sync.dma_start(out=dbg_ext["z"][:], in_=zsum)

    # ---- attention @ V ----
    out_view = exts["out"][:].rearrange("(p two) v -> p two v", two=2)
    for j0 in range(2):
        av_ps = psV.tile([P, DV], F32, name="av_ps", tag="av")
        for kh in range(2):
            tp = psA.tile([P, 256], BF16, name="ps_et", tag="ps_m")
            nc.tensor.matmul(
                tp[:, 0:P],
                lhsT=e[:, j0, kh * P:(kh + 1) * P],
                rhs=ident_bf,
                is_transpose=True,
                start=True,
                stop=True,
            )
            eT = etp.tile([P, P], BF16, name="eT", tag="eT")
            nc.vector.tensor_copy(eT, tp[:, 0:P])
            nc.tensor.matmul(
                av_ps, lhsT=eT, rhs=v_bf[kh],
                start=(kh == 0), stop=(kh == 1),
            )
        outF = work.tile([P, DV], F32, name=f"outF{j0}", tag=f"outF{j0}")
        nc.vector.tensor_scalar_mul(outF, av_ps, zr[:, j0:j0 + 1])
        nc.sync.dma_start(out=out_view[:, j0, :], in_=outF)


@functools.lru_cache(maxsize=4)
def _get_nc(reps=1):
    return build_nc(reps=reps)


def _in_maps(inputs):
    in_maps = []
    for i in range(N_CORES):
        in_maps.append({
            "queries": np.ascontiguousarray(inputs["queries"][i], dtype=np.float32),
            "keys": np.ascontiguousarray(inputs["keys"][i], dtype=np.float32),
            "values": np.ascontiguousarray(inputs["values"][i], dtype=np.float32),
            "W_q": np.ascontiguousarray(inputs["W_q"], dtype=np.float32),
            "W_k": np.ascontiguousarray(inputs["W_k"], dtype=np.float32),
            "w_v": np.ascontiguousarray(inputs["w_v"], dtype=np.float32),
        })
    return in_maps


def _run(inputs, trace=False):
    nc = _get_nc()
    in_maps = _in_maps(inputs)
    res = run_bass_kernel_spmd(nc, in_maps, core_ids=list(range(N_CORES)), trace=trace)
    out = np.stack([res.results[i]["out"] for i in range(N_CORES)], axis=0)
    return out.astype(np.float32), res


def kernel(**inputs) -> np.ndarray:
    return _run(inputs)[0]


# revision 40
# speedup vs baseline: 1.0140x; 1.0048x over previous
"""Additive (Bahdanau) attention on 8 TRN2 NeuronCores, data-parallel over batch.

Per core (one batch b):
  qf = queries @ W_q            [Q, H]   (kept transposed: qfT [H, Q], fp32)
  kf = keys @ W_k               [K, H]   (kfT [H, K], bf16)
  scores[q, k] = sum_h w_v[h] * tanh(qf[q, h] + kf[k, h])
  out = softmax_k(scores) @ values

Engine mapping (per 16-query group, 16 groups):
  - VectorE: feat[h, k] = kfB + qfT[:, q] per-partition-scalar adds (bf16 for
    the fast DVE modes, one instr per (query, h-chunk)), the psum->sbuf score
    drains, and the final 1/Z normalization.
  - ScalarE: tanh over [128, 16*256] bf16 tiles - the throughput wall: 16.7M
    tanh/core at 1 elem/lane/cycle @ 1.2 GHz is a ~109 us floor - plus one
    exp-with-accum_out pass for softmax (max-subtraction skipped: |scores| <=
    sum|w_v| ~ 8, safely inside fp32 exp range).
  - TensorE: input transposes, projections, the w_v reduction as 4-way
    column-tiled mat-vecs over bf16 moving data (fp32 moving costs 4
    cycles/column on TRN2, bf16 costs 1), and attention @ V. Query pair
    p = 4b + j uses array column strip j (tile_position=(0, 32j), stationary =
    w_v chunk replicated to 32 cols) and psum bank b, one N=512 matmul per
    (pair, h-chunk); per strip w0 serves both banks before swapping to w1.
  - Scores land replicated on psum rows 32j..32j+31, are copy-drained to SBUF
    (bf16) and compacted by SBUF->SBUF DMAs into a dense [q-pair, (q%2, k)]
    layout so softmax and attention@V run on full 128-partition tiles.

Approximate cost-model timeline: ~144 us/core; ScalarE-bound.
"""

import functools
import sys

import numpy as np

sys.path.insert(0, "/opt/trn_rl_repo")

import concourse.bass as bass  # noqa: E402
import concourse.tile as tile  # noqa: E402
from concourse import bacc, mybir  # noqa: E402
from concourse.bass_utils import run_bass_kernel_spmd  # noqa: E402
from concourse.masks import make_identity  # noqa: E402

B, Q, K, D, H, DV = 8, 256, 256, 256, 256, 512
P = 128
GQ = 16         # queries per score sub-group (fixed: 8 pairs x 2 banks)
TGQ = 16        # queries per tanh/adds group (16 or 32)
XFUSE = 0       # of each group's GQ queries, how many use the fused bias-tanh path
TANH_SPLIT = 1  # activations per (chunk, group) big-tanh (overlap granularity)
DRAIN_MODE = "dve2"  # "dve2": DVE copy drains + end exp/accum; "act", "dve", "alt"
DMA_Q = "sync"  # queue for compaction DMAs: "sync", "scalar", "gpsimd", "alt"
GPS_ADDS = 0    # how many of each group's GQ adds (per chunk) go to GPSIMD
SKEW = 0        # software-pipeline the drain by one group
STAGE_F32 = 0   # stage/compaction in f32 (v2 behavior) instead of bf16
CASTS_GPS = 1   # input bf16 casts on gpsimd instead of DVE
TRUNC = 0       # 0 full; 1 no softmax/AV; 2 no drains; 3 adds+tanh only; 4 adds only
MM_ORDER = "jpair"  # "pair" | "jpair" (weights shared across banks) | "pass"
SC_SPLIT = 1    # scores psum as two per-bank tiles (finer drain pipelining)
BUFS = dict(featp=2, tanhp=2, stagep=3, etp=2, psA=2, psS=2, psV=2)
NG = Q // GQ    # number of groups
F32 = mybir.dt.float32
BF16 = mybir.dt.bfloat16
AF = mybir.ActivationFunctionType
N_CORES = 8


def build_nc(dbg=False, reps=1):
    assert not (dbg and reps != 1)
    nc = bacc.Bacc("TRN2", target_bir_lowering=False, debug=False)

    q_ext = nc.declare_dram_parameter("queries", [Q, D], F32, isOutput=False)
    k_ext = nc.declare_dram_parameter("keys", [K, D], F32, isOutput=False)
    v_ext = nc.declare_dram_parameter("values", [K, DV], F32, isOutput=False)
    wq_ext = nc.declare_dram_parameter("W_q", [D, H], F32, isOutput=False)
    wk_ext = nc.declare_dram_parameter("W_k", [D, H], F32, isOutput=False)
    wv_ext = nc.declare_dram_parameter("w_v", [H], F32, isOutput=False)
    out_ext = nc.declare_dram_parameter("out", [Q, DV], F32, isOutput=True)
    dbg_ext = {}
    if dbg:
        dbg_ext["qfT"] = nc.declare_dram_parameter("dbg_qfT", [2, P, Q], F32, isOutput=True)
        dbg_ext["scoresD"] = nc.declare_dram_parameter("dbg_scoresD", [P, 2, K], F32, isOutput=True)
        dbg_ext["z"] = nc.declare_dram_parameter("dbg_z", [P, 2], F32, isOutput=True)
        dbg_ext["stage"] = nc.declare_dram_parameter("dbg_stage", [P, 2, 512], F32, isOutput=True)

    with tile.TileContext(nc) as tc:
        with (
            tc.tile_pool(name="consts", bufs=1) as consts,
            tc.tile_pool(name="io", bufs=1) as io,
            tc.tile_pool(name="work", bufs=1) as work,
            tc.tile_pool(name="featp", bufs=BUFS["featp"]) as featp,
            tc.tile_pool(name="tanhp", bufs=BUFS["tanhp"]) as tanhp,
            tc.tile_pool(name="stagep", bufs=BUFS["stagep"]) as stagep,
            tc.tile_pool(name="etp", bufs=BUFS["etp"]) as etp,
            tc.tile_pool(name="psA", bufs=BUFS["psA"], space=bass.MemorySpace.PSUM) as psA,
            tc.tile_pool(name="psS", bufs=BUFS["psS"], space=bass.MemorySpace.PSUM) as psS,
            tc.tile_pool(name="psV", bufs=BUFS["psV"], space=bass.MemorySpace.PSUM) as psV,
        ):
            ident = consts.tile([P, P], F32)
            make_identity(nc, ident)
            ident_bf = consts.tile([P, P], BF16)
            make_identity(nc, ident_bf)
            ident = (ident, ident_bf)
            pools = dict(consts=consts, io=io, work=work, featp=featp,
                         tanhp=tanhp, stagep=stagep, etp=etp,
                         psA=psA, psS=psS, psV=psV)
            exts = dict(q=q_ext, k=k_ext, v=v_ext, wq=wq_ext, wk=wk_ext,
                        wv=wv_ext, out=out_ext)
            for _rep in range(reps):
                _kernel_body(nc, pools, exts, ident, dbg_ext)

    nc.compile()
    return nc


def _kernel_body(nc, pools, exts, ident, dbg_ext):
    io, work, consts = pools["io"], pools["work"], pools["consts"]
    featp, tanhp, stagep, etp = (pools["featp"], pools["tanhp"],
                                 pools["stagep"], pools["etp"])
    psA, psS, psV = pools["psA"], pools["psS"], pools["psV"]
    ident, ident_bf = ident
    dbg = bool(dbg_ext)

    # ---- input loads (keys path first: it gates the first feat adds) ----
    qin, kin, v_sb, wq_sb, wk_sb = [], [], [], [], []
    for t in range(2):
        kt = io.tile([P, D], F32, name=f"kin{t}", tag=f"kin{t}")
        nc.sync.dma_start(out=kt, in_=exts["k"][t * P:(t + 1) * P, :])
        kin.append(kt)
        wkt = io.tile([P, H], F32, name=f"wk{t}", tag=f"wk{t}")
        nc.sync.dma_start(out=wkt, in_=exts["wk"][t * P:(t + 1) * P, :])
        wk_sb.append(wkt)
    for t in range(2):
        qt = io.tile([P, D], F32, name=f"qin{t}", tag=f"qin{t}")
        nc.sync.dma_start(out=qt, in_=exts["q"][t * P:(t + 1) * P, :])
        qin.append(qt)
        wqt = io.tile([P, H], F32, name=f"wq{t}", tag=f"wq{t}")
        nc.sync.dma_start(out=wqt, in_=exts["wq"][t * P:(t + 1) * P, :])
        wq_sb.append(wqt)

    # bf16 casts of matmul operands
    v_bf, wq_bf, wk_bf = [], [], []
    for t in range(2):
        wkb = io.tile([P, H], BF16, name=f"wkbf{t}", tag=f"wkbf{t}")
        (nc.gpsimd if CASTS_GPS else nc.vector).tensor_copy(out=wkb, in_=wk_sb[t])
        wk_bf.append(wkb)
    for t in range(2):
        wqb = io.tile([P, H], BF16, name=f"wqbf{t}", tag=f"wqbf{t}")
        (nc.gpsimd if CASTS_GPS else nc.vector).tensor_copy(out=wqb, in_=wq_sb[t])
        wq_bf.append(wqb)

    wv_sb = consts.tile([P, 2], F32, name="wv_sb", tag="wv_sb")
    for c in range(2):
        nc.sync.dma_start(out=wv_sb[:, c:c + 1], in_=exts["wv"][c * P:(c + 1) * P])
    # w_v chunks replicated to 32 bf16 columns: stationary for the matvecs
    wv_rep = consts.tile([P, 2, 32], BF16, name="wv_rep", tag="wv_rep")
    for c in range(2):
        nc.gpsimd.tensor_copy(
            out=wv_rep[:, c, :],
            in_=wv_sb[:, c:c + 1].broadcast_to((P, 32)),
        )

    # ---- transpose queries/keys -> bf16 [d_sub, q] ----
    qT = [work.tile([P, Q], BF16, name=f"qTd{dc}", tag=f"qTd{dc}") for dc in range(2)]
    kT = [work.tile([P, K], BF16, name=f"kTd{dc}", tag=f"kTd{dc}") for dc in range(2)]
    for src_tiles, dstT in ((kin, kT), (qin, qT)):
        for dc in range(2):
            for t in range(2):
                tp = psA.tile([P, 256], F32, name="ps_tr", tag="ps_m")
                nc.tensor.matmul(
                    tp[:, 0:P],
                    lhsT=src_tiles[t][:, dc * P:(dc + 1) * P],
                    rhs=ident,
                    is_transpose=True,
                    start=True,
                    stop=True,
                )
                nc.vector.tensor_copy(dstT[dc][:, t * P:(t + 1) * P], tp[:, 0:P])

    # ---- projections: qfT[c] f32 (bias source), kfB[c] bf16 (add source) ----
    qfT, kfB = [], []
    for name, srcT, w_tiles in (("kf", kT, wk_bf), ("qf", qT, wq_bf)):
        for c in range(2):
            pp = psA.tile([P, 256], F32, name="ps_pr", tag="ps_m")
            for dc in range(2):
                nc.tensor.matmul(
                    pp,
                    lhsT=w_tiles[dc][:, c * P:(c + 1) * P],
                    rhs=srcT[dc],
                    start=(dc == 0),
                    stop=(dc == 1),
                )
            if name == "qf":
                t_sb = work.tile([P, Q], F32, name=f"qfT{c}", tag=f"qfT{c}")
                nc.vector.tensor_copy(t_sb, pp)
                qfT.append(t_sb)
            else:
                t_bf = work.tile([P, K], BF16, name=f"kfB{c}", tag=f"kfB{c}")
                nc.vector.tensor_copy(t_bf, pp)
                kfB.append(t_bf)

    if dbg:
        for c in range(2):
            nc.sync.dma_start(out=dbg_ext["qfT"][c], in_=qfT[c])

    # values load + bf16 cast (only needed by the AV tail; off the head path)
    for t in range(2):
        vt = io.tile([P, DV], F32, name=f"vin{t}", tag=f"vin{t}")
        nc.sync.dma_start(out=vt, in_=exts["v"][t * P:(t + 1) * P, :])
        v_sb.append(vt)
        vb = io.tile([P, DV], BF16, name=f"vbf{t}", tag=f"vbf{t}")
        (nc.gpsimd if CASTS_GPS else nc.vector).tensor_copy(out=vb, in_=v_sb[t])
        v_bf.append(vb)

    # ---- main loop over query groups (drain software-pipelined one group) ----
    # eD[p, j0, k] = exp(scores[2p + j0, k]); exp happens in the psum drain
    eD = work.tile([P, 2, K], BF16, name="eD", tag="eD")
    pend = None  # (g, sc_ps) awaiting drain

    def drain(g, sc_ps):
        # drain = exp: every psum row holds real scores (32 replicated rows
        # per strip). Groups alternate between an ACT exp-drain (e values) and
        # a DVE copy-drain (raw scores, exp'd once at the end) to balance the
        # two engines; copy-drained groups write the dense tile sD instead.
        is_act = DRAIN_MODE == "act" or (DRAIN_MODE == "alt" and g % 2 == 0)
        if DRAIN_MODE == "dve2":
            is_act = False
        st = stagep.tile([P, 2, 512], F32 if STAGE_F32 else BF16,
                         name="stage", tag="stage")
        if isinstance(sc_ps, tuple):
            for b in range(2):
                if is_act:
                    nc.scalar.activation(out=st[:, b, :], in_=sc_ps[b][:, 0, :], func=AF.Exp)
                else:
                    nc.vector.tensor_copy(out=st[:, b, :], in_=sc_ps[b][:, 0, :])
        elif is_act:
            nc.scalar.activation(out=st, in_=sc_ps, func=AF.Exp)
        else:
            nc.vector.tensor_copy(out=st, in_=sc_ps)
        if dbg and g == 0:
            nc.gpsimd.dma_start(out=dbg_ext["stage"][:], in_=st)
        # compact rows {0,32,64,96} -> eD/sD[8g:8g+8]; pair p=4b+j lands at
        # partition 8g+p holding (q_even | q_odd) halves. One DMA per bank b
        # (SBUF DMA APs may only cross partitions on their first dim); the
        # two HWDGE queues (sync, act) alternate by group.
        dst = eD if is_act else sD
        dq = {"sync": nc.sync, "scalar": nc.scalar, "gpsimd": nc.gpsimd}.get(
            DMA_Q, [nc.sync, nc.scalar][g % 2])
        for b in range(2):
            dq.dma_start(
                out=dst[8 * g + 4 * b:8 * g + 4 * b + 4, :, :],
                in_=st[0:P:32, b, :],
            )

    sD = work.tile([P, 2, K], F32 if STAGE_F32 else BF16, name="sD", tag="sD")
    tanh_big = None
    for g in range(NG):
        # adds + tanh emitted once per TGQ queries; score sub-groups are 16
        if (g * GQ) % TGQ == 0:
            tanh_big = []
            for c in range(2):
                nv = TGQ - XFUSE
                th = tanhp.tile([P, TGQ * K], BF16, name=f"tanh{c}", tag=f"tanh{c}")
                if nv:
                    feat = featp.tile([P, nv * K], BF16, name=f"feat{c}", tag=f"feat{c}")
                    for qi in range(nv):
                        q = (g * GQ // TGQ) * TGQ + qi
                        eng = nc.gpsimd if qi < GPS_ADDS else nc.vector
                        eng.tensor_scalar_add(
                            out=feat[:, qi * K:(qi + 1) * K],
                            in0=kfB[c],
                            scalar1=qfT[c][:, q:q + 1],
                        )
                    step = (nv * K) // TANH_SPLIT
                    for si in range(TANH_SPLIT if TRUNC < 4 else 0):
                        nc.scalar.activation(
                            out=th[:, si * step:(si + 1) * step],
                            in_=feat[:, si * step:(si + 1) * step],
                            func=AF.Tanh,
                        )
                for qi in range(nv, TGQ):
                    q = (g * GQ // TGQ) * TGQ + qi
                    nc.scalar.activation(
                        out=th[:, qi * K:(qi + 1) * K],
                        in_=kfB[c],
                        func=AF.Tanh,
                        bias=qfT[c][:, q:q + 1],
                    )
                tanh_big.append(th)
        off = (g * GQ) % TGQ
        tanh_t = [tb[:, off * K:(off + GQ) * K] for tb in tanh_big]

        if TRUNC >= 3:
            continue
        # scores: pair p=4b+j covers queries (16g+2p, 16g+2p+1); strip j,
        # psum bank b, rows 32j..32j+31, one N=512 matmul per (pair, chunk)
        if SC_SPLIT:
            sc_b0 = psS.tile([P, 1, 512], F32, name="sc_b0", tag="sc_b0")
            sc_b1 = psS.tile([P, 1, 512], F32, name="sc_b1", tag="sc_b1")
            sc_parts = (sc_b0, sc_b1)
        else:
            sc_ps = psS.tile([P, 2, 512], F32, name="sc_ps", tag="sc")
            sc_parts = None
        if MM_ORDER == "jpair":
            # per strip: w0 once for both banks, then w1 for both banks.
            # Bank-granular has_written clears make this safe: each bank sees
            # start -> accumulate before any other start touches it.
            for j in range(4):
                for c in range(2):
                    for b in range(2):
                        p = 4 * b + j
                        if sc_parts is not None:
                            o = sc_parts[b][32 * j:32 * j + 32, 0, :]
                        else:
                            o = sc_ps[32 * j:32 * j + 32, b, :]
                        mv = slice(2 * p * K, (2 * p + 2) * K)
                        nc.tensor.matmul(
                            o, lhsT=wv_rep[:, c, :], rhs=tanh_t[c][:, mv],
                            start=(c == 0), stop=(c == 1),
                            tile_position=(0, 32 * j),
                        )
        elif MM_ORDER == "pass":
            for c in range(2):
                for j in range(4):
                    for b in range(2):
                        p = 4 * b + j
                        o = sc_ps[32 * j:32 * j + 32, b, :]
                        mv = slice(2 * p * K, (2 * p + 2) * K)
                        nc.tensor.matmul(
                            o, lhsT=wv_rep[:, c, :], rhs=tanh_t[c][:, mv],
                            start=(c == 0), stop=(c == 1),
                            tile_position=(0, 32 * j),
                        )
        else:
            for b in range(2):
                for j in range(4):
                    p = 4 * b + j
                    o = sc_ps[32 * j:32 * j + 32, b, :]
                    mv = slice(2 * p * K, (2 * p + 2) * K)
                    nc.tensor.matmul(
                        o, lhsT=wv_rep[:, 0, :], rhs=tanh_t[0][:, mv],
                        start=True, stop=False, tile_position=(0, 32 * j),
                    )
                    nc.tensor.matmul(
                        o, lhsT=wv_rep[:, 1, :], rhs=tanh_t[1][:, mv],
                        start=False, stop=True, tile_position=(0, 32 * j),
                    )

        if TRUNC >= 2:
            continue
        sc_handle = sc_parts if sc_parts is not None else sc_ps
        if SKEW:
            if pend is not None:
                drain(*pend)
            pend = (g, sc_handle)
        else:
            drain(g, sc_handle)
    if pend is not None and TRUNC < 2:
        drain(*pend)

    # exp the copy-drained groups' scores (odd groups live at partition
    # ranges [8g, 8g+8) of sD); finish them into eD in two activation calls
    # covering the odd-group partition stripes via a strided partition AP is
    # not possible on ACT, so do one activation per odd group stripe.
    if DRAIN_MODE == "dve2":
        pass  # exp+accum happens in the softmax section below
    elif DRAIN_MODE != "act":
        gs = range(1, NG, 2) if DRAIN_MODE == "alt" else range(NG)
        for g in gs:
            nc.scalar.activation(
                out=eD[8 * g:8 * g + 8, :, :],
                in_=sD[8 * g:8 * g + 8, :, :],
                func=AF.Exp,
            )


    if TRUNC >= 1:
        # still emit an output so the graph has one
        dummy = work.tile([P, DV], F32, name="dummy_out", tag="outF0")
        nc.vector.memset(dummy, 0.0)
        ov = exts["out"][:].rearrange("(p two) v -> p two v", two=2)
        nc.sync.dma_start(out=ov[:, 0, :], in_=dummy)
        return

    # ---- softmax denominator from the dense e tile ----
    e = eD
    zsum = work.tile([P, 2], F32, name="zsum", tag="zsum")
    if DRAIN_MODE == "dve2":
        for j0 in range(2):
            nc.scalar.activation(
                out=eD[:, j0, :],
                in_=sD[:, j0, :],
                func=AF.Exp,
                accum_out=zsum[:, j0:j0 + 1],
            )
    else:
        for j0 in range(2):
            nc.vector.reduce_sum(
                out=zsum[:, j0:j0 + 1], in_=eD[:, j0, :], axis=mybir.AxisListType.X
            )
    zr = work.tile([P, 2], F32, name="zr", tag="zr")
    nc.vector.reciprocal(zr, zsum)
    if dbg:
        nc.gpsimd.dma_start(out=dbg_ext["scoresD"][:], in_=eD)
        nc.sync.dma_start(out=dbg_ext["z"][:], in_=zsum)

    # ---- attention @ V ----
    out_view = exts["out"][:].rearrange("(p two) v -> p two v", two=2)
    for j0 in range(2):
        av_ps = psV.tile([P, DV], F32, name="av_ps", tag="av")
        for kh in range(2):
            tp = psA.tile([P, 256], BF16, name="ps_et", tag="ps_m")
            nc.tensor.matmul(
                tp[:, 0:P],
                lhsT=e[:, j0, kh * P:(kh + 1) * P],
                rhs=ident_bf,
                is_transpose=True,
                start=True,
                stop=True,
            )
            eT = etp.tile([P, P], BF16, name="eT", tag="eT")
            nc.vector.tensor_copy(eT, tp[:, 0:P])
            nc.tensor.matmul(
                av_ps, lhsT=eT, rhs=v_bf[kh],
                start=(kh == 0), stop=(kh == 1),
            )
        outF = work.tile([P, DV], F32, name=f"outF{j0}", tag=f"outF{j0}")
        nc.vector.tensor_scalar_mul(outF, av_ps, zr[:, j0:j0 + 1])
        nc.sync.dma_start(out=out_view[:, j0, :], in_=outF)


@functools.lru_cache(maxsize=4)
def _get_nc(reps=1):
    return build_nc(reps=reps)


def _in_maps(inputs):
    in_maps = []
    for i in range(N_CORES):
        in_maps.append({
            "queries": np.ascontiguousarray(inputs["queries"][i], dtype=np.float32),
            "keys": np.ascontiguousarray(inputs["keys"][i], dtype=np.float32),
            "values": np.ascontiguousarray(inputs["values"][i], dtype=np.float32),
            "W_q": np.ascontiguousarray(inputs["W_q"], dtype=np.float32),
            "W_k": np.ascontiguousarray(inputs["W_k"], dtype=np.float32),
            "w_v": np.ascontiguousarray(inputs["w_v"], dtype=np.float32),
        })
    return in_maps


def _run(inputs, trace=False):
    nc = _get_nc()
    in_maps = _in_maps(inputs)
    res = run_bass_kernel_spmd(nc, in_maps, core_ids=list(range(N_CORES)), trace=trace)
    out = np.stack([res.results[i]["out"] for i in range(N_CORES)], axis=0)
    return out.astype(np.float32), res


def kernel(**inputs) -> np.ndarray:
    return _run(inputs)[0]
